# revision 1
# baseline (speedup 1.0000x reference)
"""Trainium2 Bass kernel for nn_Encoder_71528385347709 (gnn_message_passing).

3-layer TransformerConv (heads=1) GNN encoder + per-layer global mean pool.

Distribution: nodes sharded graph-contiguously across 8 NeuronCores (8 graphs
each, batch_ids sorted); edges assigned to the dst-owning core; per-layer halo
exchange of node states via a shared-output AllGather.  Per-edge work reduces
to a single gather of the source-node state h_src (512B rows) through
    alpha_e = (q @ Wk^T)_dst . h_src  (+ per-dst const, softmax-invariant)
    agg_dst = (sum_e softmax(alpha)_e h_src_e) @ Wv + bv
so no per-edge k/v is ever materialized.  Gathers use dma_gather (int16
indices) over the two halves of the replicated node table.
"""
import sys
import numpy as np

sys.path.insert(0, '/opt/trn_rl_repo')

import concourse.bass as bass              # noqa: E402
import concourse.tile as tile              # noqa: E402
from concourse import bacc, mybir          # noqa: E402
from concourse.masks import make_identity  # noqa: E402
import concourse.bass_utils as bass_utils  # noqa: E402

F32 = mybir.dt.float32
I16 = mybir.dt.int16
ALU = mybir.AluOpType
AXL = mybir.AxisListType
ACTF = mybir.ActivationFunctionType

NCORES = 8
C = 128
DEPTH = 3
B_GRAPHS = 64
GCOLS = 32          # max slot-grid columns resident per group
CALLCOLS = 8        # slot-grid columns per dma_gather call (NI <= 1024)
NEG = -1.0e30


# ---------------------------------------------------------------- host prep
def preprocess(edge_index, batch_ids, n_graphs=B_GRAPHS):
    src = np.asarray(edge_index[0], np.int64)
    dst = np.asarray(edge_index[1], np.int64)
    bid = np.asarray(batch_ids, np.int64)
    N = bid.shape[0]
    Etot = src.shape[0]
    gpc = n_graphs // NCORES

    bounds = np.searchsorted(bid, np.arange(NCORES + 1) * gpc)
    L = np.diff(bounds)
    NB = int(np.ceil((L.max() + 1) / 128.0))
    PL = NB * 128
    NF = NCORES * PL
    A_HI = min(NF, 32768)
    B_LO = max(0, NF - 32768)
    assert NF - B_LO <= 32768

    indeg = np.bincount(dst, minlength=N)
    dev_row = np.empty(N, np.int64)
    perms = []
    for c in range(NCORES):
        n0, n1 = int(bounds[c]), int(bounds[c + 1])
        order = np.argsort(indeg[n0:n1], kind='stable')
        perm = n0 + order
        perms.append(perm)
        dev_row[perm] = c * PL + np.arange(n1 - n0)

    sdev = dev_row[src]
    ddev = dev_row[dst]
    # balanced assignment: src < B_LO must be A, >= A_HI must be B, the
    # overlap window is assigned per-dst to balance list lengths
    mustA = sdev < B_LO
    flex = (~mustA) & (sdev < A_HI)
    cAm = np.bincount(ddev, weights=mustA.astype(np.float64), minlength=NF).astype(np.int64)
    nflex = np.bincount(ddev, weights=flex.astype(np.float64), minlength=NF).astype(np.int64)
    tot = np.bincount(ddev, minlength=NF).astype(np.int64)
    dA_t = np.clip((tot + 1) // 2, cAm, cAm + nflex)
    # rank of each flex edge within its dst's flex list
    keyf = ddev * 2 + (~flex)
    eof = np.argsort(keyf, kind='stable')
    fstart = np.searchsorted(keyf[eof], np.arange(NF) * 2)
    frank = np.empty(Etot, np.int64)
    frank[eof] = np.arange(Etot) - fstart[ddev[eof]]
    isA = mustA | (flex & (frank < (dA_t - cAm)[ddev]))

    cA = np.bincount(ddev, weights=isA.astype(np.float64), minlength=NF)
    cB = np.bincount(ddev, weights=(~isA).astype(np.float64), minlength=NF)
    cA = cA.astype(np.int64).reshape(NCORES, NB, 128)
    cB = cB.astype(np.int64).reshape(NCORES, NB, 128)
    DA = cA.max(axis=(0, 2))
    DB = cB.max(axis=(0, 2))
    assert int((DA + DB).max()) <= GCOLS, f"block cols {int((DA+DB).max())} > GCOLS"

    groups, cur, cur_cols = [], [], 0
    for b in range(NB):
        w = int(DA[b] + DB[b])
        if cur and cur_cols + w > GCOLS:
            groups.append(cur)
            cur, cur_cols = [], 0
        cur.append(b)
        cur_cols += w
    if cur:
        groups.append(cur)

    ginfo = []
    colA_base = np.zeros(NB, np.int64)
    colB_base = np.zeros(NB, np.int64)
    S_total = 0
    for blocks in groups:
        g = {"blocks": blocks, "col0": S_total}
        off = 0
        for b in blocks:
            colA_base[b] = S_total + off
            off += int(DA[b])
        g["ka"] = off
        for b in blocks:
            colB_base[b] = S_total + off
            off += int(DB[b])
        g["kb"] = off - g["ka"]
        g["cols"] = off
        S_total += off
        ginfo.append(g)

    dummyA = PL - 1
    dummyB = NF - 1 - B_LO
    colIsB = np.zeros(S_total, bool)
    for g in ginfo:
        colIsB[g["col0"] + g["ka"]: g["col0"] + g["cols"]] = True

    sgrid = np.where(colIsB[None, None, :], dummyB, dummyA) * np.ones(
        (NCORES, 128, 1), np.int64)
    maskg = np.full((NCORES, 128, S_total), NEG, np.float32)

    key = ddev * 2 + (~isA)
    eo = np.argsort(key, kind='stable')
    k_sorted = key[eo]
    sdev_o = sdev[eo]
    rowstartA = np.searchsorted(k_sorted, np.arange(NF) * 2)
    rowstartB = np.searchsorted(k_sorted, np.arange(NF) * 2 + 1)
    r = ddev[eo]
    c_ = r // PL
    jloc = r % PL
    b_ = jloc // 128
    p_ = jloc % 128
    e_isA = isA[eo]
    ar = np.arange(Etot)
    rankA = ar - rowstartA[r]
    rankB = ar - rowstartB[r]
    col = np.where(e_isA, colA_base[b_] + rankA, colB_base[b_] + rankB)
    val = np.where(e_isA, sdev_o, sdev_o - B_LO)
    sgrid[c_, p_, col] = val
    maskg[c_, p_, col] = 0.0
    assert sgrid.max() <= 32767 and sgrid.min() >= 0

    calls = []
    for g in ginfo:
        c0 = g["col0"]
        for a0 in range(0, g["ka"], CALLCOLS):
            calls.append((c0 + a0, min(CALLCOLS, g["ka"] - a0), False))
        for bb0 in range(0, g["kb"], CALLCOLS):
            calls.append((c0 + g["ka"] + bb0, min(CALLCOLS, g["kb"] - bb0), True))

    idx_cols = sum(8 * n for (_, n, _) in calls)
    idx16 = np.zeros((NCORES, 128, idx_cols), np.int16)
    off = 0
    call_meta = []
    for (cs, n, is_b) in calls:
        ni = 128 * n
        flat = sgrid[:, :, cs:cs + n].transpose(0, 2, 1).reshape(NCORES, ni)
        wrapped = flat.reshape(NCORES, ni // 16, 16).transpose(0, 2, 1)
        idx16[:, :, off:off + ni // 16] = np.tile(wrapped, (1, 8, 1)).astype(np.int16)
        call_meta.append({"grid_col": cs, "ncols": n, "is_b": is_b,
                          "idx_off": off, "ni": ni})
        off += ni // 16

    aux = np.zeros((NCORES, 128, NB * 10), np.float32)
    for c in range(NCORES):
        Lc = int(L[c])
        gid = bid[perms[c]] - c * gpc
        j = np.arange(Lc)
        aux[c, j % 128, (j // 128) * 10 + gid] = 1.0
        aux[c, j % 128, (j // 128) * 10 + 8] = 1.0
        aux[c, j % 128, (j // 128) * 10 + 9] = (indeg[perms[c]] > 0).astype(np.float32)

    aux2 = np.zeros((NCORES, 2, PL), np.float32)
    for c in range(NCORES):
        Lc = int(L[c])
        aux2[c, 0, :Lc] = (indeg[perms[c]] > 0).astype(np.float32)
        aux2[c, 1, :] = 1.0
    counts = np.maximum(np.bincount(bid, minlength=n_graphs), 1).astype(np.float32)
    invc = (1.0 / counts).reshape(NCORES, gpc, 1)

    return dict(NB=NB, PL=PL, NF=NF, B_LO=B_LO, aux2=aux2, groups=ginfo,
                DA=DA, DB=DB, colA_base=colA_base, colB_base=colB_base,
                call_meta=call_meta, idx16=idx16, maskg=maskg, aux=aux,
                invc=invc, perms=perms, L=L, S_total=S_total, gpc=gpc)


# ---------------------------------------------------------------- device build
def build(meta, reps=1, gather_from_shared=True, scratch=16384, nq=4,
          skip_gather=False, skip_edgedve=False, skip_exchange=False,
          fused_update=False, grouped_edge=False):
    NB, PL, NF, B_LO = meta["NB"], meta["PL"], meta["NF"], meta["B_LO"]
    S_total = meta["S_total"]
    idx_cols = meta["idx16"].shape[2]
    gpc = meta["gpc"]
    scale = float(1.0 / np.sqrt(C))

    nc = bacc.Bacc("TRN2", target_bir_lowering=False, debug=False,
                   num_devices=NCORES, dynamic_dma_scratch_size=scratch,
                   num_swdge_queues=nq)

    xT_d = nc.dram_tensor("xT", [4, PL], F32, kind="ExternalInput")
    encW_d = nc.dram_tensor("encW", [4, C - 4], F32, kind="ExternalInput")
    encbr_d = nc.dram_tensor("encbr", [128, C - 4], F32, kind="ExternalInput")
    Wq_d = nc.dram_tensor("Wq", [DEPTH, C, C], F32, kind="ExternalInput")
    WkT_d = nc.dram_tensor("WkT", [DEPTH, C, C], F32, kind="ExternalInput")
    Wv_d = nc.dram_tensor("Wv", [DEPTH, C, C], F32, kind="ExternalInput")
    Ws_d = nc.dram_tensor("Ws", [DEPTH, C, C], F32, kind="ExternalInput")
    bq_d = nc.dram_tensor("bq", [DEPTH, C, 1], F32, kind="ExternalInput")
    bvs_d = nc.dram_tensor("bvs", [2, DEPTH * C], F32, kind="ExternalInput")
    aux2_d = nc.dram_tensor("aux2", [2, PL], F32, kind="ExternalInput")
    idx_d = nc.dram_tensor("idx16", [128, idx_cols], I16, kind="ExternalInput")
    mask_d = nc.dram_tensor("maskg", [128, S_total], F32, kind="ExternalInput")
    aux_d = nc.dram_tensor("aux", [128, NB * 10], F32, kind="ExternalInput")
    invc_d = nc.dram_tensor("invc", [gpc, 1], F32, kind="ExternalInput")
    out_d = nc.dram_tensor("out", [gpc, DEPTH * C], F32, kind="ExternalOutput")

    hf_sh = [nc.dram_tensor(f"hf{l}", [NF, C], F32, addr_space="Shared")
             for l in range(DEPTH)]
    hf_loc = None if gather_from_shared else nc.dram_tensor("hfloc", [NF, C], F32)
    in_b = nc.dram_tensor("in_b", [PL, C], F32)

    with tile.TileContext(nc) as tc:
        with tc.tile_pool(name="cst", bufs=1) as cst, \
             tc.tile_pool(name="st", bufs=1) as st, \
             tc.tile_pool(name="wk", bufs=2) as wk, \
             tc.tile_pool(name="wk1", bufs=1) as wk1, \
             tc.tile_pool(name="hgp", bufs=2) as hgp, \
             tc.tile_pool(name="ps", bufs=2, space="PSUM") as ps, \
             tc.tile_pool(name="ps1", bufs=1, space="PSUM") as ps1, \
             tc.tile_pool(name="psacc", bufs=1, space="PSUM") as psacc:

            # ---- constants (loaded once)
            xT = cst.tile([4, PL], F32)
            nc.sync.dma_start(out=xT[:], in_=xT_d[:])
            encW = cst.tile([4, C - 4], F32)
            encbr = cst.tile([128, C - 4], F32)
            nc.sync.dma_start(out=encW[:], in_=encW_d[:])
            nc.sync.dma_start(out=encbr[:], in_=encbr_d[:])
            Wq = cst.tile([C, DEPTH, C], F32)
            WkT = cst.tile([C, DEPTH, C], F32)
            Wv = cst.tile([C, DEPTH, C], F32)
            Ws = cst.tile([C, DEPTH, C], F32)
            bq = cst.tile([C, DEPTH, 1], F32)
            for (t, d) in ((Wq, Wq_d), (WkT, WkT_d), (Wv, Wv_d), (Ws, Ws_d),
                           (bq, bq_d)):
                nc.sync.dma_start(out=t[:], in_=d[:].rearrange("l a b -> a l b"))
            bvs = cst.tile([2, DEPTH, C], F32)
            nc.sync.dma_start(out=bvs[:],
                              in_=bvs_d[:].rearrange("t (l c) -> t l c", l=DEPTH))
            aux2 = cst.tile([2, PL], F32)
            nc.sync.dma_start(out=aux2[:], in_=aux2_d[:])
            idx16 = cst.tile([128, idx_cols], I16)
            nc.sync.dma_start(out=idx16[:], in_=idx_d[:])
            maskg = cst.tile([128, S_total], F32)
            nc.sync.dma_start(out=maskg[:], in_=mask_d[:])
            aux = cst.tile([128, NB, 10], F32)
            nc.sync.dma_start(out=aux[:],
                              in_=aux_d[:].rearrange("p (b t) -> p b t", b=NB))
            invc = cst.tile([gpc, 1], F32)
            nc.sync.dma_start(out=invc[:], in_=invc_d[:])
            ident = cst.tile([128, 128], F32)
            make_identity(nc, ident[:])

            # ---- persistent state
            hT = st.tile([128, PL], F32)
            hnm = st.tile([128, NB, C], F32)
            qtld = st.tile([128, NB, C], F32)
            outp = st.tile([gpc, DEPTH * C], F32)

            for _rep in range(reps):
                # ===== h0 = [x, x@encW + encb], node-major, then transpose
                for b in range(NB):
                    pb = ps.tile([128, C], F32, space="PSUM", tag="pb")
                    nc.tensor.matmul(out=pb[:, 0:C - 4],
                                     lhsT=xT[:, b * 128:(b + 1) * 128],
                                     rhs=encW[:], start=True, stop=True)
                    nc.vector.tensor_tensor(
                        out=hnm[:, b, 4:C], in0=pb[:, 0:C - 4],
                        in1=encbr[:], op=ALU.add)
                    # first 4 channels: copy x rows (from xT via transpose trick:
                    # xT[:, block] is [4, 128]; transpose -> [128, 4])
                    ptr = ps.tile([128, 128], F32, space="PSUM", tag="ptr")
                    nc.tensor.transpose(out=ptr[:, 0:4],
                                        in_=xT[:, b * 128:(b + 1) * 128],
                                        identity=ident[0:4, 0:4])
                    nc.vector.tensor_copy(out=hnm[:, b, 0:4], in_=ptr[:, 0:4])
                    nc.vector.tensor_scalar(out=hnm[:, b, :], in0=hnm[:, b, :],
                                            scalar1=aux[:, b, 8:9], scalar2=None,
                                            op0=ALU.mult)
                    ptr2 = ps.tile([128, 128], F32, space="PSUM", tag="ptr")
                    nc.tensor.transpose(out=ptr2[:], in_=hnm[:, b, :],
                                        identity=ident[:])
                    nc.vector.tensor_copy(out=hT[:, b * 128:(b + 1) * 128],
                                          in_=ptr2[:])

                for l in range(DEPTH):
                    # ===== halo exchange
                    nc.sync.dma_start(
                        out=in_b[:].rearrange("(b p) c -> p b c", p=128),
                        in_=hnm[:])
                    if not skip_exchange:
                        nc.gpsimd.collective_compute(
                            "AllGather", ALU.bypass,
                            replica_groups=[list(range(NCORES))],
                            ins=[in_b[:].opt()], outs=[hf_sh[l][:].opt()])
                    if gather_from_shared:
                        hf = hf_sh[l]
                    else:
                        if not skip_exchange:
                            nc.sync.dma_start(out=hf_loc[:], in_=hf_sh[l][:])
                        hf = hf_loc

                    # ===== node phase: q, qtilde
                    for n0 in range(0, PL, 512):
                        nw = min(512, PL - n0)
                        pq = ps.tile([C, 512], F32, space="PSUM", tag="pqt")
                        nc.tensor.matmul(out=pq[:, 0:nw], lhsT=Wq[:, l, :],
                                         rhs=hT[:, n0:n0 + nw],
                                         start=True, stop=True)
                        qt = wk.tile([C, 512], F32, tag="qt")
                        nc.vector.tensor_scalar(out=qt[:, 0:nw], in0=pq[:, 0:nw],
                                                scalar1=bq[:, l, 0:1],
                                                scalar2=None, op0=ALU.add)
                        for s in range(nw // 128):
                            b = n0 // 128 + s
                            pb = ps.tile([128, C], F32, space="PSUM", tag="pb")
                            nc.tensor.matmul(out=pb[:],
                                             lhsT=qt[:, s * 128:(s + 1) * 128],
                                             rhs=WkT[:, l, :],
                                             start=True, stop=True)
                            nc.vector.tensor_scalar(
                                out=qtld[:, b, :], in0=pb[:], scalar1=scale,
                                scalar2=None, op0=ALU.mult)

                    # ===== edge + update phase, grouped
                    ppool = psacc.tile([gpc, C], F32, space="PSUM", tag="pp")
                    qrr = 0
                    for g in meta["groups"]:
                        gc0, gcols = g["col0"], g["cols"]
                        hg = hgp.tile([128, GCOLS, C], F32, tag="hg")
                        for cm in meta["call_meta"]:
                            if skip_gather:
                                break
                            if not (gc0 <= cm["grid_col"] < gc0 + gcols):
                                continue
                            lc0 = cm["grid_col"] - gc0
                            src_ap = hf[B_LO:, :] if cm["is_b"] else hf[:32768 if NF > 32768 else NF, :]
                            nc.gpsimd.dma_gather(
                                out_ap=hg[:, lc0:lc0 + cm["ncols"], :],
                                in_ap=src_ap,
                                idxs_ap=idx16[:, cm["idx_off"]:
                                              cm["idx_off"] + cm["ni"] // 16],
                                num_idxs=cm["ni"], num_idxs_reg=cm["ni"],
                                elem_size=C, queue_num=qrr % nq)
                            qrr += 1
                        use_grouped = grouped_edge and not skip_edgedve
                        ug = None
                        if use_grouped:
                            b0g, nbg = g["b0"], g["nb"]
                            Da, Db = g["Da"], g["Db"]
                            ka, kb, cols = g["ka"], g["kb"], g["cols"]
                            ug = wk.tile([128, 4, C], F32, tag="ug")
                            if cols == 0:
                                nc.vector.memset(ug[:], 0.0)
                            else:
                                wh = wk1.tile([128, GCOLS, C], F32, tag="wh")
                                alph = wk.tile([128, GCOLS], F32, tag="al")
                                if ka:
                                    nc.vector.tensor_tensor(
                                        out=wh[:, 0:ka, :].rearrange(
                                            "p (b d) c -> p b d c", b=nbg),
                                        in0=hg[:, 0:ka, :].rearrange(
                                            "p (b d) c -> p b d c", b=nbg),
                                        in1=qtld[:, b0g:b0g + nbg, :].unsqueeze(2)
                                        .to_broadcast([128, nbg, Da, C]),
                                        op=ALU.mult)
                                if kb:
                                    nc.vector.tensor_tensor(
                                        out=wh[:, ka:cols, :].rearrange(
                                            "p (b d) c -> p b d c", b=nbg),
                                        in0=hg[:, ka:cols, :].rearrange(
                                            "p (b d) c -> p b d c", b=nbg),
                                        in1=qtld[:, b0g:b0g + nbg, :].unsqueeze(2)
                                        .to_broadcast([128, nbg, Db, C]),
                                        op=ALU.mult)
                                nc.vector.tensor_reduce(
                                    out=alph[:, 0:cols], in_=wh[:, 0:cols, :],
                                    axis=AXL.X, op=ALU.add)
                                nc.vector.tensor_tensor(
                                    out=alph[:, 0:cols], in0=alph[:, 0:cols],
                                    in1=maskg[:, gc0:gc0 + cols], op=ALU.add)
                                mneg = wk.tile([128, 4], F32, tag="mn2")
                                mB2 = wk.tile([128, 4], F32, tag="mb2")
                                if ka:
                                    nc.vector.tensor_reduce(
                                        out=mneg[:, 0:nbg],
                                        in_=alph[:, 0:ka].rearrange(
                                            "p (b d) -> p b d", b=nbg),
                                        axis=AXL.X, op=ALU.max, negate=True)
                                if kb:
                                    tgt = mB2 if ka else mneg
                                    nc.vector.tensor_reduce(
                                        out=tgt[:, 0:nbg],
                                        in_=alph[:, ka:cols].rearrange(
                                            "p (b d) -> p b d", b=nbg),
                                        axis=AXL.X, op=ALU.max, negate=True)
                                    if ka:
                                        nc.vector.tensor_tensor(
                                            out=mneg[:, 0:nbg], in0=mneg[:, 0:nbg],
                                            in1=mB2[:, 0:nbg], op=ALU.min)
                                if ka:
                                    nc.vector.tensor_tensor(
                                        out=alph[:, 0:ka].rearrange(
                                            "p (b d) -> p b d", b=nbg),
                                        in0=alph[:, 0:ka].rearrange(
                                            "p (b d) -> p b d", b=nbg),
                                        in1=mneg[:, 0:nbg].unsqueeze(2)
                                        .to_broadcast([128, nbg, Da]),
                                        op=ALU.add)
                                if kb:
                                    nc.vector.tensor_tensor(
                                        out=alph[:, ka:cols].rearrange(
                                            "p (b d) -> p b d", b=nbg),
                                        in0=alph[:, ka:cols].rearrange(
                                            "p (b d) -> p b d", b=nbg),
                                        in1=mneg[:, 0:nbg].unsqueeze(2)
                                        .to_broadcast([128, nbg, Db]),
                                        op=ALU.add)
                                ex = wk.tile([128, GCOLS], F32, tag="ex")
                                nc.scalar.activation(
                                    out=ex[:, 0:cols], in_=alph[:, 0:cols],
                                    func=ACTF.Exp)
                                ssum = wk.tile([128, 4], F32, tag="ss2")
                                sB2 = wk.tile([128, 4], F32, tag="sb2")
                                if ka:
                                    nc.vector.tensor_reduce(
                                        out=ssum[:, 0:nbg],
                                        in_=ex[:, 0:ka].rearrange(
                                            "p (b d) -> p b d", b=nbg),
                                        axis=AXL.X, op=ALU.add)
                                if kb:
                                    tgt = sB2 if ka else ssum
                                    nc.vector.tensor_reduce(
                                        out=tgt[:, 0:nbg],
                                        in_=ex[:, ka:cols].rearrange(
                                            "p (b d) -> p b d", b=nbg),
                                        axis=AXL.X, op=ALU.add)
                                    if ka:
                                        nc.vector.tensor_tensor(
                                            out=ssum[:, 0:nbg], in0=ssum[:, 0:nbg],
                                            in1=sB2[:, 0:nbg], op=ALU.add)
                                rcp = wk.tile([128, 4], F32, tag="rc2")
                                nc.vector.reciprocal(out=rcp[:, 0:nbg],
                                                     in_=ssum[:, 0:nbg])
                                if ka:
                                    nc.vector.tensor_tensor(
                                        out=wh[:, 0:ka, :].rearrange(
                                            "p (b d) c -> p b d c", b=nbg),
                                        in0=hg[:, 0:ka, :].rearrange(
                                            "p (b d) c -> p b d c", b=nbg),
                                        in1=ex[:, 0:ka].rearrange(
                                            "p (b d) -> p b d", b=nbg).unsqueeze(3)
                                        .to_broadcast([128, nbg, Da, C]),
                                        op=ALU.mult)
                                if kb:
                                    nc.vector.tensor_tensor(
                                        out=wh[:, ka:cols, :].rearrange(
                                            "p (b d) c -> p b d c", b=nbg),
                                        in0=hg[:, ka:cols, :].rearrange(
                                            "p (b d) c -> p b d c", b=nbg),
                                        in1=ex[:, ka:cols].rearrange(
                                            "p (b d) -> p b d", b=nbg).unsqueeze(3)
                                        .to_broadcast([128, nbg, Db, C]),
                                        op=ALU.mult)
                                ugB = wk.tile([128, 4, C], F32, tag="ug2")
                                if ka:
                                    nc.vector.tensor_reduce(
                                        out=ug[:, 0:nbg, :],
                                        in_=wh[:, 0:ka, :].rearrange(
                                            "p (b d) c -> p b c d", b=nbg),
                                        axis=AXL.X, op=ALU.add)
                                if kb:
                                    tgt = ugB if ka else ug
                                    nc.vector.tensor_reduce(
                                        out=tgt[:, 0:nbg, :],
                                        in_=wh[:, ka:cols, :].rearrange(
                                            "p (b d) c -> p b c d", b=nbg),
                                        axis=AXL.X, op=ALU.add)
                                    if ka:
                                        nc.vector.tensor_tensor(
                                            out=ug[:, 0:nbg, :], in0=ug[:, 0:nbg, :],
                                            in1=ugB[:, 0:nbg, :], op=ALU.add)
                                nc.vector.tensor_tensor(
                                    out=ug[:, 0:nbg, :], in0=ug[:, 0:nbg, :],
                                    in1=rcp[:, 0:nbg].unsqueeze(2)
                                    .to_broadcast([128, nbg, C]), op=ALU.mult)
                        for b in g["blocks"]:
                            da, db = int(meta["DA"][b]), int(meta["DB"][b])
                            dt = da + db
                            u_b = None
                            if use_grouped:
                                pass
                            elif dt == 0 or skip_edgedve:
                                u_b = wk.tile([128, C], F32, tag="ub")
                                nc.vector.memset(u_b[:], 0.0)
                            else:
                                u_b = wk.tile([128, C], F32, tag="ub")
                                _dummy = 0
                            if u_b is not None and not skip_edgedve and dt > 0:
                                a0 = int(meta["colA_base"][b]) - gc0
                                b0 = int(meta["colB_base"][b]) - gc0
                                wh = wk1.tile([128, GCOLS, C], F32, tag="wh")
                                alph = wk.tile([128, GCOLS], F32, tag="al")
                                if da:
                                    nc.vector.tensor_tensor(
                                        out=wh[:, 0:da, :],
                                        in0=hg[:, a0:a0 + da, :],
                                        in1=qtld[:, b, :].unsqueeze(1)
                                        .to_broadcast([128, da, C]),
                                        op=ALU.mult)
                                if db:
                                    nc.vector.tensor_tensor(
                                        out=wh[:, da:dt, :],
                                        in0=hg[:, b0:b0 + db, :],
                                        in1=qtld[:, b, :].unsqueeze(1)
                                        .to_broadcast([128, db, C]),
                                        op=ALU.mult)
                                nc.vector.tensor_reduce(
                                    out=alph[:, 0:dt], in_=wh[:, 0:dt, :],
                                    axis=AXL.X, op=ALU.add)
                                if da:
                                    nc.vector.tensor_tensor(
                                        out=alph[:, 0:da], in0=alph[:, 0:da],
                                        in1=maskg[:, gc0 + a0:gc0 + a0 + da],
                                        op=ALU.add)
                                if db:
                                    nc.vector.tensor_tensor(
                                        out=alph[:, da:dt], in0=alph[:, da:dt],
                                        in1=maskg[:, gc0 + b0:gc0 + b0 + db],
                                        op=ALU.add)
                                nmax = wk.tile([128, 1], F32, tag="nm")
                                nc.vector.tensor_reduce(
                                    out=nmax[:], in_=alph[:, 0:dt],
                                    axis=AXL.X, op=ALU.max, negate=True)
                                ex = wk.tile([128, GCOLS], F32, tag="ex")
                                ssum = wk.tile([128, 1], F32, tag="ss")
                                nc.scalar.activation(
                                    out=ex[:, 0:dt], in_=alph[:, 0:dt],
                                    func=ACTF.Exp, bias=nmax[:, 0:1],
                                    scale=1.0, accum_out=ssum[:])
                                rcp = wk.tile([128, 1], F32, tag="rc")
                                nc.vector.reciprocal(out=rcp[:], in_=ssum[:])
                                if da:
                                    nc.vector.tensor_tensor(
                                        out=wh[:, 0:da, :],
                                        in0=hg[:, a0:a0 + da, :],
                                        in1=ex[:, 0:da].unsqueeze(2)
                                        .to_broadcast([128, da, C]),
                                        op=ALU.mult)
                                if db:
                                    nc.vector.tensor_tensor(
                                        out=wh[:, da:dt, :],
                                        in0=hg[:, b0:b0 + db, :],
                                        in1=ex[:, da:dt].unsqueeze(2)
                                        .to_broadcast([128, db, C]),
                                        op=ALU.mult)
                                nc.vector.tensor_reduce(
                                    out=u_b[:],
                                    in_=wh[:, 0:dt, :].rearrange("p d c -> p c d"),
                                    axis=AXL.X, op=ALU.add)
                                nc.vector.tensor_scalar(
                                    out=u_b[:], in0=u_b[:], scalar1=rcp[:, 0:1],
                                    scalar2=None, op0=ALU.mult)
                            # --- per-block update
                            u_src = ug[:, b - g["b0"], :] if use_grouped else u_b[:]
                            ptr = ps.tile([128, 128], F32, space="PSUM", tag="ptr")
                            nc.tensor.transpose(out=ptr[:], in_=u_src,
                                                identity=ident[:])
                            uTb = wk.tile([128, 128], F32, tag="uTb")
                            nc.vector.tensor_copy(out=uTb[:], in_=ptr[:])
                            psk = ps.tile([128, C], F32, space="PSUM", tag="pb")
                            if fused_update:
                                nc.tensor.matmul(out=psk[:],
                                                 lhsT=hT[:, b * 128:(b + 1) * 128],
                                                 rhs=Ws[:, l, :], start=True, stop=False)
                                nc.tensor.matmul(out=psk[:],
                                                 lhsT=aux2[:, b * 128:(b + 1) * 128],
                                                 rhs=bvs[:, l, :], start=False, stop=False)
                                nc.tensor.matmul(out=psk[:], lhsT=uTb[:],
                                                 rhs=Wv[:, l, :], start=False, stop=True)
                                nc.vector.tensor_scalar(out=hnm[:, b, :], in0=psk[:],
                                                        scalar1=aux[:, b, 8:9],
                                                        scalar2=None, op0=ALU.mult)
                            else:
                                nc.tensor.matmul(out=psk[:],
                                                 lhsT=hT[:, b * 128:(b + 1) * 128],
                                                 rhs=Ws[:, l, :], start=True, stop=False)
                                nc.tensor.matmul(out=psk[:],
                                                 lhsT=aux2[:, b * 128:(b + 1) * 128],
                                                 rhs=bvs[:, l, :], start=False, stop=True)
                                pag = ps1.tile([128, C], F32, space="PSUM", tag="pb2")
                                nc.tensor.matmul(out=pag[:], lhsT=uTb[:],
                                                 rhs=Wv[:, l, :], start=True, stop=True)
                                nc.vector.tensor_copy(out=hnm[:, b, :], in_=psk[:])
                                nc.vector.tensor_tensor(out=hnm[:, b, :],
                                                        in0=hnm[:, b, :], in1=pag[:],
                                                        op=ALU.add)
                                nc.vector.tensor_scalar(out=hnm[:, b, :],
                                                        in0=hnm[:, b, :],
                                                        scalar1=aux[:, b, 8:9],
                                                        scalar2=None, op0=ALU.mult)
                            nc.tensor.matmul(out=ppool[:], lhsT=aux[:, b, 0:gpc],
                                             rhs=hnm[:, b, :], start=(b == 0),
                                             stop=(b == NB - 1))
                            ptr2 = ps.tile([128, 128], F32, space="PSUM", tag="ptr")
                            nc.tensor.transpose(out=ptr2[:], in_=hnm[:, b, :],
                                                identity=ident[:])
                            nc.vector.tensor_copy(
                                out=hT[:, b * 128:(b + 1) * 128], in_=ptr2[:])
                    nc.vector.tensor_scalar(out=outp[:, l * C:(l + 1) * C],
                                            in0=ppool[:], scalar1=invc[:, 0:1],
                                            scalar2=None, op0=ALU.mult)

            nc.sync.dma_start(out=out_d[:], in_=outp[:])
    nc.compile()
    return nc


# ---------------------------------------------------------------- input maps
def input_maps(meta, x, enc_W, enc_b, Wq, bq, Wk, bk, Wv, bv, Ws, bs):
    PL = meta["PL"]
    in_maps = []
    WkT = np.ascontiguousarray(np.transpose(np.asarray(Wk, np.float32), (0, 2, 1)))
    bvs = np.stack([np.asarray(bv, np.float32),
                    np.asarray(bs, np.float32)], axis=1)
    for c in range(NCORES):
        perm = meta["perms"][c]
        Lc = int(meta["L"][c])
        xp = np.zeros((PL, 4), np.float32)
        xp[:Lc] = np.asarray(x, np.float32)[perm]
        in_maps.append({
            "xT": np.ascontiguousarray(xp.T),
            "encW": np.asarray(enc_W, np.float32),
            "encbr": np.tile(np.asarray(enc_b, np.float32).reshape(1, -1), (128, 1)),
            "Wq": np.asarray(Wq, np.float32),
            "WkT": WkT,
            "Wv": np.asarray(Wv, np.float32),
            "Ws": np.asarray(Ws, np.float32),
            "bq": np.asarray(bq, np.float32).reshape(DEPTH, C, 1),
            "bvs": bvs.transpose(1, 0, 2).reshape(2, -1).copy(),
            "aux2": meta["aux2"][c],
            "idx16": meta["idx16"][c],
            "maskg": meta["maskg"][c],
            "aux": meta["aux"][c].reshape(128, -1),
            "invc": meta["invc"][c],
        })
    return in_maps


def assemble_output(meta, results, n_graphs=B_GRAPHS):
    gpc = meta["gpc"]
    out = np.zeros((n_graphs, DEPTH * C), np.float32)
    for c in range(NCORES):
        out[c * gpc:(c + 1) * gpc] = results[c]["out"]
    return out


_CACHE = {}


def kernel(x, edge_index, batch_ids, enc_W, enc_b, Wq, bq, Wk, bk, Wv, bv, Ws, bs):
    key = (np.asarray(x).shape, np.asarray(edge_index).tobytes()[:64],
           np.asarray(batch_ids).tobytes()[:64])
    if key not in _CACHE:
        meta = preprocess(np.asarray(edge_index), np.asarray(batch_ids))
        nc = build(meta, reps=1)
        _CACHE[key] = (meta, nc)
    meta, nc = _CACHE[key]
    in_maps = input_maps(meta, x, enc_W, enc_b, Wq, bq, Wk, bk, Wv, bv, Ws, bs)
    res = bass_utils.run_bass_kernel_spmd(nc, in_maps, core_ids=list(range(NCORES)))
    return assemble_output(meta, res.results)

